# revision 62
# baseline (speedup 1.0000x reference)
"""DarkCapsuleNet on 8 Trainium2 NeuronCores.

Data-parallel over batch (B=8, one image per core). The conv+BN+LReLU
backbone runs per core on its image; BN batch statistics are combined
across cores with tiny AllReduces (per-channel [mean, E[x^2]] sums). The
capsule-routing stage is independent per (grid-cell, image), so each core
routes its own 16 cells entirely in SBUF.

Convs are direct convolutions: matmuls accumulated over kernel offsets with
input channels on the contraction dim, bf16 operands, fp32 PSUM. Priors use
a block-diagonal lhsT built on-chip with one masked DVE multiply per tile,
so the 8-wide capsule contraction still runs as full 128-wide matmuls.
"""

import numpy as np
import ml_dtypes


class _PhaseStop(Exception):
    def __init__(self, nc):
        self.nc = nc

N_CLASSES = 43
KO = N_CLASSES * 21  # 903
EPS = 1e-5
NCORES = 8

# engine-assignment knobs (units of 1/16), tuned against TimelineSim
PM_EXP = 7    # premult tiles via Act-expand + packed DVE multiply
PM_DVE = 12   # ...up to this: DVE broadcast multiply; rest Pool
DM_DVE = 6    # delta-mult tiles on DVE; rest Pool
RED_PAIR = 0  # delta-reduce tiles with Pool pair-add pre-fold
CP_ACT = 11   # P_t PSUM->SBUF copies on Act; rest DVE
CP_SPLIT = 1  # split each P_t copy Act/DVE column-wise

_BF16 = ml_dtypes.bfloat16


# ---------------------------------------------------------------------------
# Workaround: this walrus build accepts at most ONE sem wait on a TPB_CTRL
# Drain instruction; Tile's epilogue drain carries one wait per HW-DMA queue.
# Split the extra waits onto standalone SP nops (same engine, before the
# all-engine barrier, so semantics are unchanged).
# ---------------------------------------------------------------------------
def _install_tile_drain_fix():
    import concourse.tile as tile_mod
    import concourse.mybir as mybir
    from concourse.vector_clock import ScopedClock

    if getattr(tile_mod.TileContext, "_drain_fix_installed", False):
        return

    def _patched(self, tick_clock, wait_clock):
        drain_inst = self.nc.sync.drain()
        wait_clock.add_sem_waits(
            drain_inst.ins, ScopedClock({None: tick_clock.global_clock})
        )
        raw = drain_inst.ins
        si = getattr(raw, "sync_info", None)
        if si is not None and si.on_wait is not None and len(si.on_wait) > 1:
            waits = list(si.on_wait)
            si.on_wait = waits[-1:]
            for w in waits[:-1]:
                nop = self.nc.sync.nop(nofuse=True, hint="split_drain_wait")
                nsi = getattr(nop.ins, "sync_info", None)
                if nsi is None:
                    nop.ins.sync_info = mybir.SyncInfo(on_update=[], on_wait=[w])
                else:
                    nw = list(nsi.on_wait) if nsi.on_wait else []
                    nw.append(w)
                    nsi.on_wait = nw
        self.nc.all_engine_barrier()
        assert self.sems is not None
        popped = self.nc._tile_sem_poison_stack.pop()
        assert popped is self._sem_poison
        self.nc.clear_and_free_semaphores(list(self.sems.allocated().values()))
        self.nc.all_engine_barrier()

    tile_mod.TileContext._drain_and_barrier = _patched
    tile_mod.TileContext._drain_fix_installed = True


# ---------------------------------------------------------------------------
# Host-side layout prep
# ---------------------------------------------------------------------------
def _bf(x):
    return np.ascontiguousarray(np.asarray(x, np.float32).astype(_BF16))


_F8 = ml_dtypes.float8_e4m3
W8_SCALE = 32.0  # fp8 weight prescale; BN renormalization cancels it
A8_SCALE = 16.0  # fp8 activation prescale, folded into the BN affine


def _f8w(x):
    return np.ascontiguousarray(
        (np.asarray(x, np.float32) * W8_SCALE).astype(_F8))


def _im2col(img):
    # img (3,128,128) f32 -> (27,16384), rows (ci,ky,kx)
    xp = np.zeros((3, 130, 130), np.float32)
    xp[:, 1:129, 1:129] = img
    cols = np.empty((3, 3, 3, 128, 128), np.float32)
    for ky in range(3):
        for kx in range(3):
            cols[:, ky, kx] = xp[:, ky : ky + 128, kx : kx + 128]
    return cols.reshape(27, 16384)


def _prep_shared(d):
    c1h = np.asarray(d["c1w"], np.float32).reshape(128, 27).T.copy()
    c2h = np.asarray(d["c2w"], np.float32).transpose(2, 3, 1, 0).reshape(9, 128, 256)
    c2h = np.concatenate(list(c2h), axis=1)  # (128, 9*256)
    c3t = np.asarray(d["c3w"], np.float32).transpose(1, 2, 3, 0)  # (256,4,4,64)
    c3h = np.concatenate(
        [c3t[m * 128 : (m + 1) * 128].reshape(128, 16 * 64) for m in range(2)], axis=1
    )  # (128, 2048)
    c4h = np.asarray(d["c4w"], np.float32).transpose(1, 2, 3, 0).reshape(64, 16 * 128)
    c5h = np.asarray(d["c5w"], np.float32).transpose(1, 2, 3, 0).reshape(128, 16 * 256)

    rw = np.asarray(d["rw"], np.float32)  # (512,43,8,21)
    rt = rw.transpose(0, 2, 1, 3).reshape(512 * 8, KO)  # row = n*8+i
    # RT[t*128 + ns*8 + i] = rw[16t+ns, :, i, :]  -> same as rt row (16t+ns)*8+i
    # rt rows are already (n,i) with n major: n*8+i = (16t+ns)*8+i = t*128+ns*8+i ✓

    gb = np.zeros((128, 14), np.float32)
    gb[:, 0] = d["g1"]; gb[:, 1] = d["b1"]
    gb[:, 2] = d["g2"][:128]; gb[:, 3] = d["b2"][:128]
    gb[:, 4] = d["g2"][128:]; gb[:, 5] = d["b2"][128:]
    gb[:64, 6] = d["g3"]; gb[:64, 7] = d["b3"]
    gb[:, 8] = d["g4"]; gb[:, 9] = d["b4"]
    gb[:, 10] = d["g5"][:128]; gb[:, 11] = d["b5"][:128]
    gb[:, 12] = d["g5"][128:]; gb[:, 13] = d["b5"][128:]

    mask = np.zeros((128, 128), np.float32)
    for p in range(128):
        mask[p, (p >> 3) * 8 : (p >> 3) * 8 + 8] = 1.0
    selb = np.zeros((128, 8), np.float32)
    for p in range(128):
        selb[p, p & 7] = 1.0
    selr = np.zeros((8, 128), np.float32)  # [b, ns*8 + b]
    for ns in range(16):
        for b in range(8):
            selr[b, ns * 8 + b] = 1.0
    return dict(
        c1wT=_bf(c1h), c2wT=_bf(c2h), c3wT=_bf(c3h), c4wT=_bf(c4h), c5wT=_bf(c5h),
        RT=_bf(rt), gb=gb, MASK=_bf(mask), SELB=_bf(selb), SELB43=_bf(selb / 43.0),
        SELR=_bf(selr),
    )


# ---------------------------------------------------------------------------
# Bass program (identical on every core)
# ---------------------------------------------------------------------------
def _spill_extra_waits(nc):
    """This walrus codegen accepts at most one semaphore wait per TPB
    instruction. Tile can attach several. Move the extras onto fresh NoOp
    instructions inserted just before the owner on the same engine."""
    import concourse.mybir as mybir

    uid = [0]
    for f in nc.m.functions:
        for bb in f.blocks:
            il = bb.instructions
            out = []
            changed = False
            for inst in il:
                si = getattr(inst, "sync_info", None)
                waits = list(si.on_wait) if si is not None and si.on_wait else []
                if len(waits) > 1:
                    for w in waits[:-1]:
                        uid[0] += 1
                        nop = mybir.InstNoOp(name=f"waitspill-{uid[0]}", ins=[], outs=[])
                        nop.engine = inst.engine
                        nop.sync_info = mybir.SyncInfo(on_update=[], on_wait=[w])
                        out.append(nop)
                    si.on_wait = waits[-1:]
                    changed = True
                out.append(inst)
            if changed:
                bb.instructions = out


def _build_bass(phase_limit=99):
    import concourse.bass as bass
    import concourse.mybir as mybir
    from concourse import tile

    _install_tile_drain_fix()

    F32 = mybir.dt.float32
    BF16 = mybir.dt.bfloat16
    F16 = mybir.dt.float16
    F8 = mybir.dt.float8e4
    ADD = mybir.AluOpType.add
    MULT = mybir.AluOpType.mult
    SUB = mybir.AluOpType.subtract
    ACTF = mybir.ActivationFunctionType
    AXX = mybir.AxisListType.X

    nc = bass.Bass(num_devices=NCORES)
    dp = nc.declare_dram_parameter
    i_xcol = dp("xcol", [27, 16384], BF16, isOutput=False)
    i_c1 = dp("c1wT", [27, 128], BF16, isOutput=False)
    i_c2 = dp("c2wT", [128, 2304], BF16, isOutput=False)
    i_c3 = dp("c3wT", [128, 2048], BF16, isOutput=False)
    i_c4 = dp("c4wT", [64, 2048], BF16, isOutput=False)
    i_c5 = dp("c5wT", [128, 4096], BF16, isOutput=False)
    i_rt = dp("RT", [4096, KO], BF16, isOutput=False)
    i_gb = dp("gb", [128, 14], F32, isOutput=False)
    i_mask = dp("MASK", [128, 128], BF16, isOutput=False)
    i_selb = dp("SELB", [128, 8], BF16, isOutput=False)
    i_selb43 = dp("SELB43", [128, 8], BF16, isOutput=False)
    i_selr = dp("SELR", [8, 128], BF16, isOutput=False)
    o_out = dp("out", [16, KO], F32, isOutput=True)


    with tile.TileContext(nc) as tc:
        with tc.tile_pool(name="const", bufs=1) as const, \
             tc.tile_pool(name="dram", bufs=1, space="DRAM") as dram:
            t_gb = const.tile([128, 14], F32)
            t_mask = const.tile([128, 128], BF16)
            t_selb = const.tile([128, 8], BF16)
            t_selb43 = const.tile([128, 8], BF16)
            t_selr = const.tile([8, 128], BF16)
            h5 = [const.tile([128, 256], BF16, tag=f"h5_{m}", name=f"h5_{m}") for m in range(2)]
            t_st6 = const.tile([128, 32 * 6], F32)
            t_mv = const.tile([128, 4], F32)
            t_ab = const.tile([128, 4], F32)
            t_sc = const.tile([128, 8], F32)
            for t, i in [(t_gb, i_gb), (t_mask, i_mask), (t_selb, i_selb),
                         (t_selb43, i_selb43), (t_selr, i_selr)]:
                nc.sync.dma_start(t[:], i[:])

            # BN cross-core sync: 8 AllGather slots per sync (one per sync id).
            # AllGather (bypass) avoids the cost model's 1.875x AllReduce
            # multiplier; the 8-way sum happens locally on DVE afterwards.
            NSYNC = 6  # conv2 syncs per half; conv5 one wide sync (slot 5)
            SYNCW = (2, 2, 2, 2, 2, 4)
            ar_in = [dram.tile([128, SYNCW[i]], F32, tag=f"ari{i}", name=f"ari{i}")
                     for i in range(NSYNC)]
            ar_out = [dram.tile([8, 128 * SYNCW[i]], F32, tag=f"aro{i}",
                                name=f"aro{i}") for i in range(NSYNC)]
            t_g16 = [const.tile([128, 8 * SYNCW[i]], F32, tag=f"g16_{i}",
                                name=f"g16_{i}") for i in range(NSYNC)]

            def bn_sync_start(sync, mcol, npart, ncols=1):
                """t_mv[:, 2*(mcol+k)] = local mean, [.., +1] = local var for
                each of ncols channel-groups; push [m, E[x^2]] pairs through
                AllGather slot `sync`."""
                w = 2 * ncols
                for k in range(ncols):
                    m = t_mv[:npart, 2 * (mcol + k) : 2 * (mcol + k) + 1]
                    v = t_mv[:npart, 2 * (mcol + k) + 1 : 2 * (mcol + k) + 2]
                    s1 = t_sc[:npart, sync : sync + 1]
                    nc.vector.tensor_tensor(s1, m, m, MULT)
                    nc.vector.tensor_tensor(v, v, s1, ADD)  # v := E[x^2] local
                nc.sync.dma_start(ar_in[sync][:],
                                  t_mv[:, 2 * mcol : 2 * mcol + w])
                nc.gpsimd.collective_compute(
                    "AllGather", mybir.AluOpType.bypass,
                    ins=[ar_in[sync][:]], outs=[ar_out[sync][:]],
                    replica_groups=[list(range(NCORES))],
                )
                # gathered block r (core r's [128,w]) is flat [128w*r, ...)
                # = ar_out[r, w*p+c]; land it in SBUF as column group w*r+c.
                src = ar_out[sync][:].rearrange("r (p c) -> p r c", c=w)
                nc.sync.dma_start(
                    t_g16[sync][:].rearrange("p (r c) -> p r c", c=w), src)

            def bn_sync_reduce(sync, abcol, npart, ncols=1):
                w = 2 * ncols
                g = t_g16[sync][:npart, :].rearrange("p (r c) -> p c r", c=w)
                nc.vector.tensor_reduce(
                    t_mv[:npart, 2 * abcol : 2 * abcol + w], g, AXX, ADD)

            def bn_sync_finish(sync, gcol, abcol, npart, skip_reduce=False):
                """Sum the 8 gathered [m, Ex2] pairs, finalize affine into
                t_ab[:, 2*abcol:2*abcol+2]."""
                m = t_mv[:npart, 2 * abcol : 2 * abcol + 1]
                q = t_mv[:npart, 2 * abcol + 1 : 2 * abcol + 2]
                if not skip_reduce:
                    bn_sync_reduce(sync, abcol, npart)
                a = t_ab[:npart, 2 * abcol : 2 * abcol + 1]
                b = t_ab[:npart, 2 * abcol + 1 : 2 * abcol + 2]
                s1 = t_sc[:npart, sync : sync + 1]
                nc.vector.tensor_scalar_mul(m, m, 1.0 / NCORES)
                nc.vector.tensor_scalar_mul(q, q, 1.0 / NCORES)
                nc.scalar.activation(s1, m, ACTF.Square)
                nc.vector.tensor_tensor(q, q, s1, SUB)       # gvar
                nc.vector.tensor_scalar_add(q, q, EPS)
                nc.vector.reciprocal(s1, q)
                nc.scalar.activation(s1, s1, ACTF.Sqrt)      # rsqrt(var+eps)
                nc.vector.tensor_tensor(a, t_gb[:npart, gcol : gcol + 1], s1, MULT)
                nc.vector.tensor_tensor(s1, a, m, MULT)
                nc.vector.tensor_tensor(b, t_gb[:npart, gcol + 1 : gcol + 2], s1, SUB)

            def bn_allreduce(layer, nch_tiles, npart, sync0=None):
                syncs = {0: 0, 1: 1, 2: 3, 3: 4, 4: 5}[layer] if sync0 is None else sync0
                for mt in range(nch_tiles):
                    bn_sync_start(syncs + mt, mt, npart)
                for mt in range(nch_tiles):
                    gcol = (0, 2, 6, 8, 10)[layer] + 2 * mt
                    bn_sync_finish(syncs + mt, gcol, mt, npart)

            def lrelu_apply(view, scale, bias, out=None):
                nc.scalar.activation(view if out is None else out, view,
                                     ACTF.Prelu, bias=bias, scale=scale,
                                     alpha=0.1)

            # ================= conv backbone =================
            with tc.tile_pool(name="wpool", bufs=1) as wp, \
                 tc.tile_pool(name="xpool", bufs=1) as xp, \
                 tc.tile_pool(name="acts", bufs=1) as acts, \
                 tc.tile_pool(name="cpsum", bufs=8, space="PSUM") as cpsum:
                t_c2 = wp.tile([128, 2304], BF16)
                t_c3 = wp.tile([128, 2048], BF16)
                t_c4 = wp.tile([64, 2048], BF16)
                t_c5 = wp.tile([128, 4096], BF16)
                t_c1 = xp.tile([27, 128], BF16)
                t_xcol = xp.tile([27, 16384], BF16)
                nc.sync.dma_start(t_c1[:], i_c1[:])
                for ch in range(4):
                    nc.sync.dma_start(t_xcol[:, ch * 4096 : (ch + 1) * 4096],
                                      i_xcol[:, ch * 4096 : (ch + 1) * 4096])

                h1 = acts.tile([128, 130 * 130], BF16)
                h2 = [acts.tile([128, 130 * 130], BF16, tag=f"h2_{m}", name=f"h2_{m}") for m in range(2)]
                h3 = acts.tile([64, 66 * 66], BF16)
                h4 = acts.tile([128, 34 * 34], BF16)

                def zero_border(tile_ap, H):
                    v = tile_ap.rearrange("p (a b) -> p a b", b=H)
                    nc.gpsimd.memset(v[:, 0:1, :], 0.0)
                    nc.gpsimd.memset(v[:, H - 1 : H, :], 0.0)
                    nc.gpsimd.memset(v[:, 1 : H - 1, 0:1], 0.0)
                    nc.gpsimd.memset(v[:, 1 : H - 1, H - 1 : H], 0.0)

                zero_border(h1[:], 130)
                zero_border(h2[0][:], 130)
                zero_border(h2[1][:], 130)
                zero_border(h3[:], 66)
                zero_border(h4[:], 34)

                # ---- conv1 ----
                for nt in range(32):
                    ps = cpsum.tile([128, 512], F32, tag="cps")
                    nc.tensor.matmul(ps[:], t_c1[:],
                                     t_xcol[:, nt * 512 : (nt + 1) * 512],
                                     start=True, stop=True)
                    intr = h1[:].rearrange("p (a b) -> p a b", b=130)[
                        :, 1 + nt * 4 : 5 + nt * 4, 1:129]
                    nc.scalar.activation(
                        intr, ps[:].rearrange("p (a b) -> p a b", b=128), ACTF.Copy)
                    nc.vector.bn_stats(t_st6[:, nt * 6 : nt * 6 + 6], ps[:])
                for t, i in [(t_c2, i_c2), (t_c3, i_c3), (t_c4, i_c4),
                             (t_c5, i_c5)]:
                    nc.sync.dma_start(t[:], i[:])
                nc.vector.bn_aggr(t_mv[:, 0:2],
                                  t_st6[:].rearrange("p (g s) -> p g s", s=6))
                bn_allreduce(0, 1, 128)
                h1v = h1[:].rearrange("p (a b) -> p a b", b=130)
                for r0, r1 in ((1, 7), (7, 33), (33, 81), (81, 129)):
                    lrelu_apply(h1v[:, r0:r1, 1:129],
                                t_ab[:, 0:1], t_ab[:, 1:2])

                # ---- conv2 ----
                if phase_limit < 2:
                    raise _PhaseStop(nc)
                for m in range(2):
                    for nt in range(32):
                        ps = cpsum.tile([128, 512], F32, tag="cps")
                        for off in range(9):
                            ky, kx = off // 3, off % 3
                            rhs = h1v[:, ky + nt * 4 : ky + nt * 4 + 4, kx : kx + 128]
                            nc.tensor.matmul(
                                ps[:],
                                t_c2[:, off * 256 + m * 128 : off * 256 + m * 128 + 128],
                                rhs, start=(off == 0), stop=(off == 8))
                        intr = h2[m][:].rearrange("p (a b) -> p a b", b=130)[
                            :, 1 + nt * 4 : 5 + nt * 4, 1:129]
                        nc.scalar.activation(
                            intr, ps[:].rearrange("p (a b) -> p a b", b=128), ACTF.Copy)
                        nc.vector.bn_stats(t_st6[:, nt * 6 : nt * 6 + 6], ps[:])
                    nc.vector.bn_aggr(t_mv[:, 2 * m : 2 * m + 2],
                                      t_st6[:].rearrange("p (g s) -> p g s", s=6))
                    # start this half's stats exchange while the other half
                    # is still on the tensor engine
                    bn_sync_start(1 + m, m, 128)
                for m in range(2):
                    bn_sync_finish(1 + m, 2 + 2 * m, m, 128)
                h2v = [h2[m][:].rearrange("p (a b) -> p a b", b=130) for m in range(2)]
                for m in range(2):
                    for r0, r1 in ((1, 17), (17, 65), (65, 129)):
                        lrelu_apply(h2v[m][:, r0:r1, 1:129],
                                    t_ab[:, 2 * m : 2 * m + 1],
                                    t_ab[:, 2 * m + 1 : 2 * m + 2])

                # ---- conv3 ----
                if phase_limit < 3:
                    raise _PhaseStop(nc)
                for nt in range(8):
                    ps = cpsum.tile([128, 512], F32, tag="cps")
                    first = True
                    for m in range(2):
                        for off in range(16):
                            ky, kx = off // 4, off % 4
                            rhs = h2v[m][:, ky + nt * 16 : ky + nt * 16 + 15 : 2,
                                         kx : kx + 127 : 2]
                            nc.tensor.matmul(
                                ps[:64, :],
                                t_c3[:, (m * 16 + off) * 64 : (m * 16 + off) * 64 + 64],
                                rhs, start=first, stop=(m == 1 and off == 15))
                            first = False
                    intr = h3[:].rearrange("p (a b) -> p a b", b=66)[
                        :, 1 + nt * 8 : 9 + nt * 8, 1:65]
                    nc.scalar.activation(
                        intr, ps[:64, :].rearrange("p (a b) -> p a b", b=64), ACTF.Copy)
                    nc.vector.bn_stats(t_st6[:64, nt * 6 : nt * 6 + 6], ps[:64, :])
                nc.vector.bn_aggr(
                    t_mv[:64, 0:2],
                    t_st6[:64, : 8 * 6].rearrange("p (g s) -> p g s", s=6))
                bn_allreduce(2, 1, 64)
                h3v = h3[:].rearrange("p (a b) -> p a b", b=66)
                for r0, r1 in ((1, 33), (33, 65)):
                    lrelu_apply(h3v[:, r0:r1, 1:65], t_ab[:64, 0:1], t_ab[:64, 1:2])

                # ---- conv4 ----
                if phase_limit < 4:
                    raise _PhaseStop(nc)
                for nt in range(2):
                    ps = cpsum.tile([128, 512], F32, tag="cps")
                    for off in range(16):
                        ky, kx = off // 4, off % 4
                        rhs = h3v[:, ky + nt * 32 : ky + nt * 32 + 31 : 2, kx : kx + 63 : 2]
                        nc.tensor.matmul(ps[:], t_c4[:, off * 128 : off * 128 + 128],
                                         rhs, start=(off == 0), stop=(off == 15))
                    intr = h4[:].rearrange("p (a b) -> p a b", b=34)[
                        :, 1 + nt * 16 : 17 + nt * 16, 1:33]
                    nc.scalar.activation(
                        intr, ps[:].rearrange("p (a b) -> p a b", b=32), ACTF.Copy)
                    nc.vector.bn_stats(t_st6[:, nt * 6 : nt * 6 + 6], ps[:])
                nc.vector.bn_aggr(
                    t_mv[:, 0:2], t_st6[:, :12].rearrange("p (g s) -> p g s", s=6))
                bn_allreduce(3, 1, 128)
                h4v = h4[:].rearrange("p (a b) -> p a b", b=34)
                lrelu_apply(h4v[:, 1:33, 1:33], t_ab[:, 0:1], t_ab[:, 1:2])

                # ---- conv5 ----
                if phase_limit < 5:
                    raise _PhaseStop(nc)
                for m in range(2):
                    ps = cpsum.tile([128, 512], F32, tag="cps")
                    first = True
                    for off in range(16):
                        ky, kx = off // 4, off % 4
                        rhs = h4v[:, ky : ky + 31 : 2, kx : kx + 31 : 2]
                        nc.tensor.matmul(
                            ps[:, 0:256],
                            t_c5[:, off * 256 + m * 128 : off * 256 + m * 128 + 128],
                            rhs, start=first, stop=(off == 15))
                        first = False
                    nc.scalar.activation(h5[m][:], ps[:, 0:256], ACTF.Copy)
                    nc.vector.bn_stats(t_st6[:, m * 6 : m * 6 + 6], ps[:, 0:256])
                    nc.vector.bn_aggr(
                        t_mv[:, 2 * m : 2 * m + 2],
                        t_st6[:, m * 6 : m * 6 + 6].rearrange("p (g s) -> p g s", s=6))
                # conv5's halves are too small to hide a collective behind;
                # one wide gather beats two
                bn_sync_start(5, 0, 128, ncols=2)
                bn_sync_reduce(5, 0, 128, ncols=2)
                for m in range(2):
                    bn_sync_finish(5, 10 + 2 * m, m, 128, skip_reduce=True)
                    lrelu_apply(h5[m][:], t_ab[:, 2 * m : 2 * m + 1],
                                t_ab[:, 2 * m + 1 : 2 * m + 2])

            if phase_limit < 6:
                raise _PhaseStop(nc)
            # ================= priors =================
            with tc.tile_pool(name="pri", bufs=1) as pri, \
                 tc.tile_pool(name="route", bufs=1) as rp, \
                 tc.tile_pool(name="scr", bufs=4) as scr:
                P = [[pri.tile([128, 8 * KO], BF16, tag=f"P{g}_{j}", name=f"P{g}_{j}")
                      for j in range(4)] for g in range(2)]

                def P_t(g, t):
                    j, tj = t // 8, t % 8
                    return P[g][j][:, tj * KO : tj * KO + KO]
                with tc.tile_pool(name="ppsum", bufs=4, space="PSUM") as ppsum:
                    for t in range(32):
                        h = t >> 3
                        w = (t >> 1) & 3
                        mblk = t & 1
                        rt_t = scr.tile([128, KO], BF16, tag="rt", bufs=8)
                        nc.sync.dma_start(rt_t[:], i_rt[t * 128 : (t + 1) * 128, :])
                        hb = h5[mblk][:].rearrange(
                            "p (hh gy gx ww) -> p hh gy gx ww",
                            hh=4, gy=4, gx=4)
                        for g in range(2):
                            g8 = scr.tile([128, 8], BF16, tag="g8")
                            src = hb[:, h : h + 1, 2 * g : 2 * g + 2, :, w : w + 1]
                            # (p,1,2,4,1) -> (p,2,4)
                            src = src.rearrange("p a b d e -> p (a b) (d e)")
                            nc.gpsimd.tensor_copy(
                                g8[:].rearrange("p (b d) -> p b d", b=2), src)
                            lt = scr.tile([128, 128], BF16, tag="lt")
                            lte = nc.vector if (t + g) % 2 == 0 else nc.gpsimd
                            lte.tensor_tensor(
                                lt[:].rearrange("p (n b) -> p n b", b=8),
                                g8[:].rearrange("p (o e) -> p o e", o=1)
                                    .broadcast_to([128, 16, 8]),
                                t_mask[:].rearrange("p (n b) -> p n b", b=8),
                                MULT)
                            pp = ppsum.tile([128, KO], F32, tag="pps")
                            nc.tensor.matmul(pp[:, 0:512], lt[:], rt_t[:, 0:512],
                                             start=True, stop=True)
                            nc.tensor.matmul(pp[:, 512:KO], lt[:], rt_t[:, 512:KO],
                                             start=True, stop=True)
                            # gpsimd cannot read PSUM; halve per-tile copy
                            # latency by splitting columns across Act and DVE
                            if CP_SPLIT:
                                nc.scalar.activation(
                                    P_t(g, t)[:, 0:512], pp[:, 0:512], ACTF.Copy)
                                nc.vector.tensor_copy(
                                    P_t(g, t)[:, 512:KO], pp[:, 512:KO])
                            elif (2 * t + g) % 16 < CP_ACT:
                                nc.scalar.activation(P_t(g, t), pp[:], ACTF.Copy)
                            else:
                                nc.vector.tensor_copy(P_t(g, t), pp[:])

                # ================= routing =================
                if phase_limit < 7:
                    raise _PhaseStop(nc)
                NG = 4   # tile-groups per cell-group (8 tiles each)
                GT = 8
                L = [[rp.tile([128, GT * 43], F16, tag=f"L{g}_{j}", name=f"L{g}_{j}")
                      for j in range(NG)] for g in range(2)]
                PR = [[rp.tile([128, GT * 43], BF16, tag=f"PR{g}_{j}", name=f"PR{g}_{j}")
                       for j in range(NG)] for g in range(2)]
                s_g = [rp.tile([8, KO], F32, tag=f"s_g{g}", name=f"s_g{g}") for g in range(2)]
                sq = [rp.tile([8, KO], F32, tag=f"sq{g}", name=f"sq{g}") for g in range(2)]
                sn = [rp.tile([8, 43], F32, tag=f"sn{g}", name=f"sn{g}") for g in range(2)]
                den = [rp.tile([8, 43], F32, tag=f"den{g}", name=f"den{g}") for g in range(2)]
                phi = [rp.tile([8, 43], F32, tag=f"phi{g}", name=f"phi{g}") for g in range(2)]
                out_f = [rp.tile([8, KO], F32, tag=f"of{g}", name=f"of{g}") for g in range(2)]
                out_bf = [rp.tile([8, KO], BF16, tag=f"ob{g}", name=f"ob{g}") for g in range(2)]
                out_rep = [rp.tile([128, KO], BF16, tag=f"orep{g}", name=f"orep{g}") for g in range(2)]
                for g in range(2):
                    for j in range(NG):
                        nc.vector.memset(L[g][j][:], 0.0)

                with tc.tile_pool(name="rpsum", bufs=2, space="PSUM") as rpsum:
                    for it in range(3):
                        for g in range(2):
                            if it > 0:
                                for j in range(NG):
                                    e8 = scr.tile([128, GT * 43], F16, tag="e8")
                                    nc.scalar.activation(e8[:], L[g][j][:], ACTF.Exp)
                                    r8 = scr.tile([128, GT], F32, tag="r8")
                                    nc.vector.tensor_reduce(
                                        r8[:], e8[:].rearrange("p (t k) -> p t k", k=43),
                                        AXX, ADD)
                                    nc.vector.reciprocal(r8[:], r8[:])
                                    nc.vector.tensor_tensor(
                                        PR[g][j][:].rearrange("p (t k) -> p t k", k=43),
                                        e8[:].rearrange("p (t k) -> p t k", k=43),
                                        r8[:].rearrange("p (t k) -> p t k", k=1)
                                            .broadcast_to([128, GT, 43]),
                                        MULT)
                            sp = rpsum.tile([8, KO], F32, tag="sps")
                            for t in range(32):
                                j, tj = t // GT, t % GT
                                if it == 0:
                                    rhs_t = P_t(g, t)
                                    lhs = t_selb43
                                else:
                                    tm = scr.tile([128, KO], BF16, tag="tm", bufs=6)
                                    pr_b = (PR[g][j][:, tj * 43 : tj * 43 + 43]
                                            .rearrange("p (k o) -> p k o", o=1)
                                            .broadcast_to([128, 43, 21]))
                                    rp_ = (2 * t + g + 3 * it) % 16
                                    if rp_ < PM_EXP:
                                        # expand probs on Act, then a packed
                                        # bf16 multiply hits DVE 2x_1p mode
                                        pre = scr.tile([128, KO], BF16,
                                                       tag="pre", bufs=3)
                                        nc.scalar.activation(
                                            pre[:].rearrange("p (k o) -> p k o", o=21),
                                            pr_b, ACTF.Copy)
                                        nc.vector.tensor_tensor(
                                            tm[:], P_t(g, t), pre[:], MULT)
                                    elif rp_ < PM_DVE:
                                        nc.vector.tensor_tensor(
                                            tm[:].rearrange("p (k o) -> p k o", o=21),
                                            P_t(g, t).rearrange("p (k o) -> p k o", o=21),
                                            pr_b, MULT)
                                    else:
                                        nc.gpsimd.tensor_tensor(
                                            tm[:].rearrange("p (k o) -> p k o", o=21),
                                            P_t(g, t).rearrange("p (k o) -> p k o", o=21),
                                            pr_b, MULT)
                                    rhs_t = tm[:]
                                    lhs = t_selb
                                nc.tensor.matmul(sp[:, 0:512], lhs[:], rhs_t[:, 0:512],
                                                 start=(t == 0), stop=(t == 31))
                                nc.tensor.matmul(sp[:, 512:KO], lhs[:], rhs_t[:, 512:KO],
                                                 start=(t == 0), stop=(t == 31))
                            nc.scalar.activation(s_g[g][:], sp[:], ACTF.Copy)
                        # squash: out = s * sqrt(sn)/(1+sn)
                        for g in range(2):
                            nc.scalar.activation(sq[g][:], s_g[g][:], ACTF.Square)
                            nc.vector.tensor_reduce(
                                sn[g][:], sq[g][:].rearrange("p (k o) -> p k o", o=21),
                                AXX, ADD)
                            nc.vector.tensor_scalar_add(den[g][:], sn[g][:], 1.0)
                            nc.vector.reciprocal(den[g][:], den[g][:])
                            nc.scalar.activation(phi[g][:], sn[g][:], ACTF.Sqrt)
                            nc.vector.tensor_tensor(phi[g][:], phi[g][:], den[g][:], MULT)
                            tgt = out_f[g] if it == 2 else out_bf[g]
                            nc.vector.tensor_tensor(
                                tgt[:].rearrange("p (k o) -> p k o", o=21),
                                s_g[g][:].rearrange("p (k o) -> p k o", o=21),
                                phi[g][:].rearrange("p (k o) -> p k o", o=1)
                                      .broadcast_to([8, 43, 21]),
                                MULT)
                            if it == 2:
                                nc.sync.dma_start(o_out[g * 8 : g * 8 + 8, :], tgt[:])
                        if it < 2:
                            for g in range(2):
                                rpp = rpsum.tile([128, KO], F32, tag="rep")
                                nc.tensor.matmul(
                                    rpp[:, 0:512], t_selr[:],
                                    out_bf[g][:, 0:512], start=True, stop=True)
                                nc.tensor.matmul(
                                    rpp[:, 512:KO], t_selr[:],
                                    out_bf[g][:, 512:KO], start=True, stop=True)
                                nc.scalar.activation(out_rep[g][:], rpp[:], ACTF.Copy)
                                for j in range(NG):
                                    arg = scr.tile([128, GT * 43], F16, tag="arg",
                                                   name="arg", bufs=2)
                                    for tj in range(GT):
                                        t = j * GT + tj
                                        ap = scr.tile([128, KO], BF16, tag="ap", bufs=6)
                                        me = (nc.vector
                                              if (2 * t + g + it) % 16 < DM_DVE
                                              else nc.gpsimd)
                                        me.tensor_tensor(
                                            ap[:], P_t(g, t), out_rep[g][:], MULT)
                                        a3 = ap[:].rearrange("p (k o) -> p k o", o=21)
                                        with nc.allow_low_precision("logit delta fp16"):
                                            if (t + g + it) % 16 < RED_PAIR:
                                                # Pool folds o-pairs so the
                                                # (DVE-only) reduce reads 11
                                                # elems per class, not 21
                                                sc = scr.tile([128, 43 * 11], BF16,
                                                              tag="sc", bufs=4)
                                                s3 = sc[:].rearrange(
                                                    "p (k q) -> p k q", q=11)
                                                nc.gpsimd.tensor_tensor(
                                                    s3[:, :, 0:10], a3[:, :, 0:20:2],
                                                    a3[:, :, 1:21:2], ADD)
                                                nc.gpsimd.tensor_copy(
                                                    s3[:, :, 10:11], a3[:, :, 20:21])
                                                nc.vector.tensor_reduce(
                                                    arg[:, tj * 43 : tj * 43 + 43],
                                                    s3, AXX, ADD)
                                            else:
                                                nc.vector.tensor_reduce(
                                                    arg[:, tj * 43 : tj * 43 + 43],
                                                    a3, AXX, ADD)
                                    nc.vector.tensor_tensor(
                                        L[g][j][:], L[g][j][:], arg[:], ADD)
    _spill_extra_waits(nc)
    return nc


_CACHED = {}


def _get_bass():
    if "nc" not in _CACHED:
        _CACHED["nc"] = _build_bass()
    return _CACHED["nc"]


def kernel(**inputs):
    from concourse.bass_utils import run_bass_kernel_spmd

    d = {k: np.asarray(v) for k, v in inputs.items()}
    shared = _prep_shared(d)
    x = np.asarray(d["x"], np.float32)

    nc = _get_bass()
    in_maps = []
    for c in range(NCORES):
        m = dict(shared)
        m["xcol"] = _bf(_im2col(x[c]))
        in_maps.append(m)

    import os
    trace = bool(os.environ.get("DCAPS_TRACE"))
    res = run_bass_kernel_spmd(
        nc, in_maps, core_ids=list(range(NCORES)), trace=trace)
    _CACHED["last_results"] = res
    _CACHED["last_in_maps"] = in_maps

    out = np.empty((NCORES, 4, 4, N_CLASSES, 21), np.float32)
    for c in range(NCORES):
        r = np.asarray(res.results[c]["out"])  # (16, 903)
        for gy in range(4):
            for gx in range(4):
                cell = (gy >> 1) * 8 + (gy & 1) * 4 + gx
                out[c, gy, gx] = r[cell].reshape(N_CLASSES, 21)
    return out



# revision 65
# speedup vs baseline: 1.0046x; 1.0046x over previous
"""DarkCapsuleNet on 8 Trainium2 NeuronCores.

Data-parallel over batch (B=8, one image per core). The conv+BN+LReLU
backbone runs per core on its image; BN batch statistics are combined
across cores with tiny AllReduces (per-channel [mean, E[x^2]] sums). The
capsule-routing stage is independent per (grid-cell, image), so each core
routes its own 16 cells entirely in SBUF.

Convs are direct convolutions: matmuls accumulated over kernel offsets with
input channels on the contraction dim, bf16 operands, fp32 PSUM. Priors use
a block-diagonal lhsT built on-chip with one masked DVE multiply per tile,
so the 8-wide capsule contraction still runs as full 128-wide matmuls.
"""

import numpy as np
import ml_dtypes


class _PhaseStop(Exception):
    def __init__(self, nc):
        self.nc = nc

N_CLASSES = 43
KO = N_CLASSES * 21  # 903
EPS = 1e-5
NCORES = 8

# engine-assignment knobs (units of 1/16), tuned against TimelineSim
PM_EXP = 7    # premult tiles via Act-expand + packed DVE multiply
PM_DVE = 12   # ...up to this: DVE broadcast multiply; rest Pool
DM_DVE = 6    # delta-mult tiles on DVE; rest Pool
RED_PAIR = 0  # delta-reduce tiles with Pool pair-add pre-fold
CP_ACT = 11   # P_t PSUM->SBUF copies on Act; rest DVE
CP_SPLIT = 1  # split each P_t copy Act/DVE column-wise

_BF16 = ml_dtypes.bfloat16


# ---------------------------------------------------------------------------
# Workaround: this walrus build accepts at most ONE sem wait on a TPB_CTRL
# Drain instruction; Tile's epilogue drain carries one wait per HW-DMA queue.
# Split the extra waits onto standalone SP nops (same engine, before the
# all-engine barrier, so semantics are unchanged).
# ---------------------------------------------------------------------------
def _install_tile_drain_fix():
    import concourse.tile as tile_mod
    import concourse.mybir as mybir
    from concourse.vector_clock import ScopedClock

    if getattr(tile_mod.TileContext, "_drain_fix_installed", False):
        return

    def _patched(self, tick_clock, wait_clock):
        drain_inst = self.nc.sync.drain()
        wait_clock.add_sem_waits(
            drain_inst.ins, ScopedClock({None: tick_clock.global_clock})
        )
        raw = drain_inst.ins
        si = getattr(raw, "sync_info", None)
        if si is not None and si.on_wait is not None and len(si.on_wait) > 1:
            waits = list(si.on_wait)
            si.on_wait = waits[-1:]
            for w in waits[:-1]:
                nop = self.nc.sync.nop(nofuse=True, hint="split_drain_wait")
                nsi = getattr(nop.ins, "sync_info", None)
                if nsi is None:
                    nop.ins.sync_info = mybir.SyncInfo(on_update=[], on_wait=[w])
                else:
                    nw = list(nsi.on_wait) if nsi.on_wait else []
                    nw.append(w)
                    nsi.on_wait = nw
        self.nc.all_engine_barrier()
        assert self.sems is not None
        popped = self.nc._tile_sem_poison_stack.pop()
        assert popped is self._sem_poison
        self.nc.clear_and_free_semaphores(list(self.sems.allocated().values()))
        self.nc.all_engine_barrier()

    tile_mod.TileContext._drain_and_barrier = _patched
    tile_mod.TileContext._drain_fix_installed = True


# ---------------------------------------------------------------------------
# Host-side layout prep
# ---------------------------------------------------------------------------
def _bf(x):
    return np.ascontiguousarray(np.asarray(x, np.float32).astype(_BF16))


_F8 = ml_dtypes.float8_e4m3
W8_SCALE = 32.0  # fp8 weight prescale; BN renormalization cancels it
A8_SCALE = 16.0  # fp8 activation prescale, folded into the BN affine


def _f8w(x):
    return np.ascontiguousarray(
        (np.asarray(x, np.float32) * W8_SCALE).astype(_F8))


def _im2col(img):
    # img (3,128,128) f32 -> (27,16384), rows (ci,ky,kx)
    xp = np.zeros((3, 130, 130), np.float32)
    xp[:, 1:129, 1:129] = img
    cols = np.empty((3, 3, 3, 128, 128), np.float32)
    for ky in range(3):
        for kx in range(3):
            cols[:, ky, kx] = xp[:, ky : ky + 128, kx : kx + 128]
    return cols.reshape(27, 16384)


def _prep_shared(d):
    c1h = np.asarray(d["c1w"], np.float32).reshape(128, 27).T.copy()
    c2h = np.asarray(d["c2w"], np.float32).transpose(2, 3, 1, 0).reshape(9, 128, 256)
    c2h = np.concatenate(list(c2h), axis=1)  # (128, 9*256)
    c3t = np.asarray(d["c3w"], np.float32).transpose(1, 2, 3, 0)  # (256,4,4,64)
    c3h = np.concatenate(
        [c3t[m * 128 : (m + 1) * 128].reshape(128, 16 * 64) for m in range(2)], axis=1
    )  # (128, 2048)
    c4h = np.asarray(d["c4w"], np.float32).transpose(1, 2, 3, 0).reshape(64, 16 * 128)
    c5h = np.asarray(d["c5w"], np.float32).transpose(1, 2, 3, 0).reshape(128, 16 * 256)

    rw = np.asarray(d["rw"], np.float32)  # (512,43,8,21)
    rt = rw.transpose(0, 2, 1, 3).reshape(512 * 8, KO)  # row = n*8+i
    # RT[t*128 + ns*8 + i] = rw[16t+ns, :, i, :]  -> same as rt row (16t+ns)*8+i
    # rt rows are already (n,i) with n major: n*8+i = (16t+ns)*8+i = t*128+ns*8+i ✓

    gb = np.zeros((128, 14), np.float32)
    gb[:, 0] = d["g1"]; gb[:, 1] = d["b1"]
    gb[:, 2] = d["g2"][:128]; gb[:, 3] = d["b2"][:128]
    gb[:, 4] = d["g2"][128:]; gb[:, 5] = d["b2"][128:]
    gb[:64, 6] = d["g3"]; gb[:64, 7] = d["b3"]
    gb[:, 8] = d["g4"]; gb[:, 9] = d["b4"]
    gb[:, 10] = d["g5"][:128]; gb[:, 11] = d["b5"][:128]
    gb[:, 12] = d["g5"][128:]; gb[:, 13] = d["b5"][128:]

    mask = np.zeros((128, 128), np.float32)
    for p in range(128):
        mask[p, (p >> 3) * 8 : (p >> 3) * 8 + 8] = 1.0
    selb = np.zeros((128, 8), np.float32)
    for p in range(128):
        selb[p, p & 7] = 1.0
    selr = np.zeros((8, 128), np.float32)  # [b, ns*8 + b]
    for ns in range(16):
        for b in range(8):
            selr[b, ns * 8 + b] = 1.0
    return dict(
        c1wT=_bf(c1h), c2wT=_bf(c2h), c3wT=_bf(c3h), c4wT=_bf(c4h), c5wT=_bf(c5h),
        RT=_bf(rt), gb=gb, MASK=_bf(mask), SELB=_bf(selb), SELB43=_bf(selb / 43.0),
        SELR=_bf(selr),
    )


# ---------------------------------------------------------------------------
# Bass program (identical on every core)
# ---------------------------------------------------------------------------
def _spill_extra_waits(nc):
    """This walrus codegen accepts at most one semaphore wait per TPB
    instruction. Tile can attach several. Move the extras onto fresh NoOp
    instructions inserted just before the owner on the same engine."""
    import concourse.mybir as mybir

    uid = [0]
    for f in nc.m.functions:
        for bb in f.blocks:
            il = bb.instructions
            out = []
            changed = False
            for inst in il:
                si = getattr(inst, "sync_info", None)
                waits = list(si.on_wait) if si is not None and si.on_wait else []
                if len(waits) > 1:
                    for w in waits[:-1]:
                        uid[0] += 1
                        nop = mybir.InstNoOp(name=f"waitspill-{uid[0]}", ins=[], outs=[])
                        nop.engine = inst.engine
                        nop.sync_info = mybir.SyncInfo(on_update=[], on_wait=[w])
                        out.append(nop)
                    si.on_wait = waits[-1:]
                    changed = True
                out.append(inst)
            if changed:
                bb.instructions = out


def _build_bass(phase_limit=99):
    import concourse.bass as bass
    import concourse.mybir as mybir
    from concourse import tile

    _install_tile_drain_fix()

    F32 = mybir.dt.float32
    BF16 = mybir.dt.bfloat16
    F16 = mybir.dt.float16
    F8 = mybir.dt.float8e4
    ADD = mybir.AluOpType.add
    MULT = mybir.AluOpType.mult
    SUB = mybir.AluOpType.subtract
    ACTF = mybir.ActivationFunctionType
    AXX = mybir.AxisListType.X

    nc = bass.Bass(num_devices=NCORES)
    dp = nc.declare_dram_parameter
    i_xcol = dp("xcol", [27, 16384], BF16, isOutput=False)
    i_c1 = dp("c1wT", [27, 128], BF16, isOutput=False)
    i_c2 = dp("c2wT", [128, 2304], BF16, isOutput=False)
    i_c3 = dp("c3wT", [128, 2048], BF16, isOutput=False)
    i_c4 = dp("c4wT", [64, 2048], BF16, isOutput=False)
    i_c5 = dp("c5wT", [128, 4096], BF16, isOutput=False)
    i_rt = dp("RT", [4096, KO], BF16, isOutput=False)
    i_gb = dp("gb", [128, 14], F32, isOutput=False)
    i_mask = dp("MASK", [128, 128], BF16, isOutput=False)
    i_selb = dp("SELB", [128, 8], BF16, isOutput=False)
    i_selb43 = dp("SELB43", [128, 8], BF16, isOutput=False)
    i_selr = dp("SELR", [8, 128], BF16, isOutput=False)
    o_out = dp("out", [16, KO], F32, isOutput=True)


    with tile.TileContext(nc) as tc:
        with tc.tile_pool(name="const", bufs=1) as const, \
             tc.tile_pool(name="dram", bufs=1, space="DRAM") as dram:
            t_gb = const.tile([128, 14], F32)
            t_mask = const.tile([128, 128], BF16)
            t_selb = const.tile([128, 8], BF16)
            t_selb43 = const.tile([128, 8], BF16)
            t_selr = const.tile([8, 128], BF16)
            h5 = [const.tile([128, 256], BF16, tag=f"h5_{m}", name=f"h5_{m}") for m in range(2)]
            t_st6 = const.tile([128, 32 * 6], F32)
            t_mv = const.tile([128, 4], F32)
            t_ab = const.tile([128, 4], F32)
            t_sc = const.tile([128, 8], F32)
            for t, i in [(t_gb, i_gb), (t_mask, i_mask), (t_selb, i_selb),
                         (t_selb43, i_selb43), (t_selr, i_selr)]:
                nc.sync.dma_start(t[:], i[:])

            # BN cross-core sync: 8 AllGather slots per sync (one per sync id).
            # AllGather (bypass) avoids the cost model's 1.875x AllReduce
            # multiplier; the 8-way sum happens locally on DVE afterwards.
            NSYNC = 6  # conv2 syncs per half; conv5 one wide sync (slot 5)
            SYNCW = (2, 2, 2, 2, 2, 4)
            ar_in = [dram.tile([128, SYNCW[i]], F32, tag=f"ari{i}", name=f"ari{i}")
                     for i in range(NSYNC)]
            ar_out = [dram.tile([8, 128 * SYNCW[i]], F32, tag=f"aro{i}",
                                name=f"aro{i}") for i in range(NSYNC)]
            t_g16 = [const.tile([128, 8 * SYNCW[i]], F32, tag=f"g16_{i}",
                                name=f"g16_{i}") for i in range(NSYNC)]

            def bn_sync_start(sync, mcol, npart, ncols=1):
                """t_mv[:, 2*(mcol+k)] = local mean, [.., +1] = local var for
                each of ncols channel-groups; push [m, E[x^2]] pairs through
                AllGather slot `sync`."""
                w = 2 * ncols
                for k in range(ncols):
                    m = t_mv[:npart, 2 * (mcol + k) : 2 * (mcol + k) + 1]
                    v = t_mv[:npart, 2 * (mcol + k) + 1 : 2 * (mcol + k) + 2]
                    s1 = t_sc[:npart, sync : sync + 1]
                    nc.vector.tensor_tensor(s1, m, m, MULT)
                    nc.vector.tensor_tensor(v, v, s1, ADD)  # v := E[x^2] local
                nc.sync.dma_start(ar_in[sync][:],
                                  t_mv[:, 2 * mcol : 2 * mcol + w])
                nc.gpsimd.collective_compute(
                    "AllGather", mybir.AluOpType.bypass,
                    ins=[ar_in[sync][:]], outs=[ar_out[sync][:]],
                    replica_groups=[list(range(NCORES))],
                )
                # gathered block r (core r's [128,w]) is flat [128w*r, ...)
                # = ar_out[r, w*p+c]; land it in SBUF as column group w*r+c.
                src = ar_out[sync][:].rearrange("r (p c) -> p r c", c=w)
                nc.sync.dma_start(
                    t_g16[sync][:].rearrange("p (r c) -> p r c", c=w), src)

            def bn_sync_reduce(sync, abcol, npart, ncols=1):
                w = 2 * ncols
                g = t_g16[sync][:npart, :].rearrange("p (r c) -> p c r", c=w)
                nc.vector.tensor_reduce(
                    t_mv[:npart, 2 * abcol : 2 * abcol + w], g, AXX, ADD)

            def bn_sync_finish(sync, gcol, abcol, npart, skip_reduce=False):
                """Sum the 8 gathered [m, Ex2] pairs, finalize affine into
                t_ab[:, 2*abcol:2*abcol+2]."""
                m = t_mv[:npart, 2 * abcol : 2 * abcol + 1]
                q = t_mv[:npart, 2 * abcol + 1 : 2 * abcol + 2]
                if not skip_reduce:
                    bn_sync_reduce(sync, abcol, npart)
                a = t_ab[:npart, 2 * abcol : 2 * abcol + 1]
                b = t_ab[:npart, 2 * abcol + 1 : 2 * abcol + 2]
                s1 = t_sc[:npart, sync : sync + 1]
                nc.vector.tensor_scalar_mul(m, m, 1.0 / NCORES)
                nc.vector.tensor_scalar_mul(q, q, 1.0 / NCORES)
                nc.scalar.activation(s1, m, ACTF.Square)
                nc.vector.tensor_tensor(q, q, s1, SUB)       # gvar
                nc.vector.tensor_scalar_add(q, q, EPS)
                nc.vector.reciprocal(s1, q)
                nc.scalar.activation(s1, s1, ACTF.Sqrt)      # rsqrt(var+eps)
                nc.vector.tensor_tensor(a, t_gb[:npart, gcol : gcol + 1], s1, MULT)
                nc.vector.tensor_tensor(s1, a, m, MULT)
                nc.vector.tensor_tensor(b, t_gb[:npart, gcol + 1 : gcol + 2], s1, SUB)

            def bn_allreduce(layer, nch_tiles, npart, sync0=None):
                syncs = {0: 0, 1: 1, 2: 3, 3: 4, 4: 5}[layer] if sync0 is None else sync0
                for mt in range(nch_tiles):
                    bn_sync_start(syncs + mt, mt, npart)
                for mt in range(nch_tiles):
                    gcol = (0, 2, 6, 8, 10)[layer] + 2 * mt
                    bn_sync_finish(syncs + mt, gcol, mt, npart)

            def lrelu_apply(view, scale, bias, out=None):
                nc.scalar.activation(view if out is None else out, view,
                                     ACTF.Prelu, bias=bias, scale=scale,
                                     alpha=0.1)

            # ================= conv backbone =================
            with tc.tile_pool(name="wpool", bufs=1) as wp, \
                 tc.tile_pool(name="xpool", bufs=1) as xp, \
                 tc.tile_pool(name="acts", bufs=1) as acts, \
                 tc.tile_pool(name="cpsum", bufs=8, space="PSUM") as cpsum:
                t_c2 = wp.tile([128, 2304], BF16)
                t_c3 = wp.tile([128, 2048], BF16)
                t_c4 = wp.tile([64, 2048], BF16)
                t_c5 = wp.tile([128, 4096], BF16)
                t_c1 = xp.tile([27, 128], BF16)
                t_xcol = xp.tile([27, 16384], BF16)
                nc.sync.dma_start(t_c1[:], i_c1[:])
                for ch in range(4):
                    nc.sync.dma_start(t_xcol[:, ch * 4096 : (ch + 1) * 4096],
                                      i_xcol[:, ch * 4096 : (ch + 1) * 4096])

                h1 = acts.tile([128, 130 * 130], BF16)
                h2 = [acts.tile([128, 130 * 130], BF16, tag=f"h2_{m}", name=f"h2_{m}") for m in range(2)]
                h3 = acts.tile([64, 66 * 66], BF16)
                h4 = acts.tile([128, 34 * 34], BF16)

                def zero_border(tile_ap, H):
                    v = tile_ap.rearrange("p (a b) -> p a b", b=H)
                    nc.gpsimd.memset(v[:, 0:1, :], 0.0)
                    nc.gpsimd.memset(v[:, H - 1 : H, :], 0.0)
                    nc.gpsimd.memset(v[:, 1 : H - 1, 0:1], 0.0)
                    nc.gpsimd.memset(v[:, 1 : H - 1, H - 1 : H], 0.0)

                zero_border(h1[:], 130)
                zero_border(h2[0][:], 130)
                zero_border(h2[1][:], 130)
                zero_border(h3[:], 66)
                zero_border(h4[:], 34)

                # ---- conv1 ----
                for nt in range(32):
                    ps = cpsum.tile([128, 512], F32, tag="cps")
                    nc.tensor.matmul(ps[:], t_c1[:],
                                     t_xcol[:, nt * 512 : (nt + 1) * 512],
                                     start=True, stop=True)
                    intr = h1[:].rearrange("p (a b) -> p a b", b=130)[
                        :, 1 + nt * 4 : 5 + nt * 4, 1:129]
                    nc.scalar.activation(
                        intr, ps[:].rearrange("p (a b) -> p a b", b=128), ACTF.Copy)
                    nc.vector.bn_stats(t_st6[:, nt * 6 : nt * 6 + 6], ps[:])
                for t, i in [(t_c2, i_c2), (t_c3, i_c3), (t_c4, i_c4),
                             (t_c5, i_c5)]:
                    nc.sync.dma_start(t[:], i[:])
                nc.vector.bn_aggr(t_mv[:, 0:2],
                                  t_st6[:].rearrange("p (g s) -> p g s", s=6))
                bn_allreduce(0, 1, 128)
                h1v = h1[:].rearrange("p (a b) -> p a b", b=130)
                for r0, r1 in ((1, 7), (7, 33), (33, 81), (81, 129)):
                    lrelu_apply(h1v[:, r0:r1, 1:129],
                                t_ab[:, 0:1], t_ab[:, 1:2])

                # ---- conv2 ----
                if phase_limit < 2:
                    raise _PhaseStop(nc)
                for m in range(2):
                    for nt in range(32):
                        ps = cpsum.tile([128, 512], F32, tag="cps")
                        for off in range(9):
                            ky, kx = off // 3, off % 3
                            rhs = h1v[:, ky + nt * 4 : ky + nt * 4 + 4, kx : kx + 128]
                            nc.tensor.matmul(
                                ps[:],
                                t_c2[:, off * 256 + m * 128 : off * 256 + m * 128 + 128],
                                rhs, start=(off == 0), stop=(off == 8))
                        intr = h2[m][:].rearrange("p (a b) -> p a b", b=130)[
                            :, 1 + nt * 4 : 5 + nt * 4, 1:129]
                        nc.scalar.activation(
                            intr, ps[:].rearrange("p (a b) -> p a b", b=128), ACTF.Copy)
                        nc.vector.bn_stats(t_st6[:, nt * 6 : nt * 6 + 6], ps[:])
                    nc.vector.bn_aggr(t_mv[:, 2 * m : 2 * m + 2],
                                      t_st6[:].rearrange("p (g s) -> p g s", s=6))
                    # start this half's stats exchange while the other half
                    # is still on the tensor engine
                    bn_sync_start(1 + m, m, 128)
                for m in range(2):
                    bn_sync_finish(1 + m, 2 + 2 * m, m, 128)
                h2v = [h2[m][:].rearrange("p (a b) -> p a b", b=130) for m in range(2)]
                for m in range(2):
                    for r0, r1 in ((1, 17), (17, 65), (65, 129)):
                        lrelu_apply(h2v[m][:, r0:r1, 1:129],
                                    t_ab[:, 2 * m : 2 * m + 1],
                                    t_ab[:, 2 * m + 1 : 2 * m + 2])

                # ---- conv3 ----
                if phase_limit < 3:
                    raise _PhaseStop(nc)
                for nt in range(8):
                    ps = cpsum.tile([128, 512], F32, tag="cps")
                    first = True
                    for m in range(2):
                        for off in range(16):
                            ky, kx = off // 4, off % 4
                            rhs = h2v[m][:, ky + nt * 16 : ky + nt * 16 + 15 : 2,
                                         kx : kx + 127 : 2]
                            nc.tensor.matmul(
                                ps[:64, :],
                                t_c3[:, (m * 16 + off) * 64 : (m * 16 + off) * 64 + 64],
                                rhs, start=first, stop=(m == 1 and off == 15))
                            first = False
                    intr = h3[:].rearrange("p (a b) -> p a b", b=66)[
                        :, 1 + nt * 8 : 9 + nt * 8, 1:65]
                    nc.scalar.activation(
                        intr, ps[:64, :].rearrange("p (a b) -> p a b", b=64), ACTF.Copy)
                    nc.vector.bn_stats(t_st6[:64, nt * 6 : nt * 6 + 6], ps[:64, :])
                nc.vector.bn_aggr(
                    t_mv[:64, 0:2],
                    t_st6[:64, : 8 * 6].rearrange("p (g s) -> p g s", s=6))
                bn_allreduce(2, 1, 64)
                h3v = h3[:].rearrange("p (a b) -> p a b", b=66)
                for r0, r1 in ((1, 33), (33, 65)):
                    lrelu_apply(h3v[:, r0:r1, 1:65], t_ab[:64, 0:1], t_ab[:64, 1:2])

                # ---- conv4 ----
                if phase_limit < 4:
                    raise _PhaseStop(nc)
                for nt in range(2):
                    ps = cpsum.tile([128, 512], F32, tag="cps")
                    for off in range(16):
                        ky, kx = off // 4, off % 4
                        rhs = h3v[:, ky + nt * 32 : ky + nt * 32 + 31 : 2, kx : kx + 63 : 2]
                        nc.tensor.matmul(ps[:], t_c4[:, off * 128 : off * 128 + 128],
                                         rhs, start=(off == 0), stop=(off == 15))
                    intr = h4[:].rearrange("p (a b) -> p a b", b=34)[
                        :, 1 + nt * 16 : 17 + nt * 16, 1:33]
                    nc.scalar.activation(
                        intr, ps[:].rearrange("p (a b) -> p a b", b=32), ACTF.Copy)
                    nc.vector.bn_stats(t_st6[:, nt * 6 : nt * 6 + 6], ps[:])
                nc.vector.bn_aggr(
                    t_mv[:, 0:2], t_st6[:, :12].rearrange("p (g s) -> p g s", s=6))
                bn_allreduce(3, 1, 128)
                h4v = h4[:].rearrange("p (a b) -> p a b", b=34)
                lrelu_apply(h4v[:, 1:33, 1:33], t_ab[:, 0:1], t_ab[:, 1:2])

                # ---- conv5 ----
                if phase_limit < 5:
                    raise _PhaseStop(nc)
                for m in range(2):
                    ps = cpsum.tile([128, 512], F32, tag="cps")
                    first = True
                    for off in range(16):
                        ky, kx = off // 4, off % 4
                        rhs = h4v[:, ky : ky + 31 : 2, kx : kx + 31 : 2]
                        nc.tensor.matmul(
                            ps[:, 0:256],
                            t_c5[:, off * 256 + m * 128 : off * 256 + m * 128 + 128],
                            rhs, start=first, stop=(off == 15))
                        first = False
                    nc.scalar.activation(h5[m][:], ps[:, 0:256], ACTF.Copy)
                    nc.vector.bn_stats(t_st6[:, m * 6 : m * 6 + 6], ps[:, 0:256])
                    nc.vector.bn_aggr(
                        t_mv[:, 2 * m : 2 * m + 2],
                        t_st6[:, m * 6 : m * 6 + 6].rearrange("p (g s) -> p g s", s=6))
                # conv5's halves are too small to hide a collective behind;
                # one wide gather beats two
                bn_sync_start(5, 0, 128, ncols=2)
                bn_sync_reduce(5, 0, 128, ncols=2)
                for m in range(2):
                    bn_sync_finish(5, 10 + 2 * m, m, 128, skip_reduce=True)
                    lrelu_apply(h5[m][:], t_ab[:, 2 * m : 2 * m + 1],
                                t_ab[:, 2 * m + 1 : 2 * m + 2])

            if phase_limit < 6:
                raise _PhaseStop(nc)
            # ================= priors =================
            with tc.tile_pool(name="pri", bufs=1) as pri, \
                 tc.tile_pool(name="route", bufs=1) as rp, \
                 tc.tile_pool(name="scr", bufs=4) as scr:
                P = [[pri.tile([128, 8 * KO], BF16, tag=f"P{g}_{j}", name=f"P{g}_{j}")
                      for j in range(4)] for g in range(2)]

                def P_t(g, t):
                    j, tj = t // 8, t % 8
                    return P[g][j][:, tj * KO : tj * KO + KO]
                with tc.tile_pool(name="ppsum", bufs=4, space="PSUM") as ppsum:
                    for t in range(32):
                        h = t >> 3
                        w = (t >> 1) & 3
                        mblk = t & 1
                        rt_t = scr.tile([128, KO], BF16, tag="rt", bufs=8)
                        nc.sync.dma_start(rt_t[:], i_rt[t * 128 : (t + 1) * 128, :])
                        hb = h5[mblk][:].rearrange(
                            "p (hh gy gx ww) -> p hh gy gx ww",
                            hh=4, gy=4, gx=4)
                        for g in range(2):
                            g8 = scr.tile([128, 8], BF16, tag="g8")
                            src = hb[:, h : h + 1, 2 * g : 2 * g + 2, :, w : w + 1]
                            # (p,1,2,4,1) -> (p,2,4)
                            src = src.rearrange("p a b d e -> p (a b) (d e)")
                            nc.gpsimd.tensor_copy(
                                g8[:].rearrange("p (b d) -> p b d", b=2), src)
                            lt = scr.tile([128, 128], BF16, tag="lt")
                            lte = nc.vector if (t + g) % 2 == 0 else nc.gpsimd
                            lte.tensor_tensor(
                                lt[:].rearrange("p (n b) -> p n b", b=8),
                                g8[:].rearrange("p (o e) -> p o e", o=1)
                                    .broadcast_to([128, 16, 8]),
                                t_mask[:].rearrange("p (n b) -> p n b", b=8),
                                MULT)
                            pp = ppsum.tile([128, KO], F32, tag="pps")
                            nc.tensor.matmul(pp[:, 0:512], lt[:], rt_t[:, 0:512],
                                             start=True, stop=True)
                            nc.tensor.matmul(pp[:, 512:KO], lt[:], rt_t[:, 512:KO],
                                             start=True, stop=True)
                            # gpsimd cannot read PSUM; halve per-tile copy
                            # latency by splitting columns across Act and DVE
                            if CP_SPLIT:
                                nc.scalar.activation(
                                    P_t(g, t)[:, 0:512], pp[:, 0:512], ACTF.Copy)
                                nc.vector.tensor_copy(
                                    P_t(g, t)[:, 512:KO], pp[:, 512:KO])
                            elif (2 * t + g) % 16 < CP_ACT:
                                nc.scalar.activation(P_t(g, t), pp[:], ACTF.Copy)
                            else:
                                nc.vector.tensor_copy(P_t(g, t), pp[:])

                # ================= routing =================
                if phase_limit < 7:
                    raise _PhaseStop(nc)
                NG = 4   # tile-groups per cell-group (8 tiles each)
                GT = 8
                L = [[rp.tile([128, GT * 43], F16, tag=f"L{g}_{j}", name=f"L{g}_{j}")
                      for j in range(NG)] for g in range(2)]
                PR = [[rp.tile([128, GT * 43], BF16, tag=f"PR{g}_{j}", name=f"PR{g}_{j}")
                       for j in range(NG)] for g in range(2)]
                s_g = [rp.tile([8, KO], F32, tag=f"s_g{g}", name=f"s_g{g}") for g in range(2)]
                sq = [rp.tile([8, KO], F32, tag=f"sq{g}", name=f"sq{g}") for g in range(2)]
                sn = [rp.tile([8, 43], F32, tag=f"sn{g}", name=f"sn{g}") for g in range(2)]
                den = [rp.tile([8, 43], F32, tag=f"den{g}", name=f"den{g}") for g in range(2)]
                phi = [rp.tile([8, 43], F32, tag=f"phi{g}", name=f"phi{g}") for g in range(2)]
                out_f = [rp.tile([8, KO], F32, tag=f"of{g}", name=f"of{g}") for g in range(2)]
                out_bf = [rp.tile([8, KO], BF16, tag=f"ob{g}", name=f"ob{g}") for g in range(2)]
                out_rep = [rp.tile([128, KO], BF16, tag=f"orep{g}", name=f"orep{g}") for g in range(2)]
                for g in range(2):
                    for j in range(NG):
                        nc.vector.memset(L[g][j][:], 0.0)

                with tc.tile_pool(name="rpsum", bufs=2, space="PSUM") as rpsum:
                    for it in range(3):
                        for g in range(2):
                            if it > 0:
                                for j in range(NG):
                                    e8 = scr.tile([128, GT * 43], F16, tag="e8")
                                    nc.scalar.activation(e8[:], L[g][j][:], ACTF.Exp)
                                    r8 = scr.tile([128, GT], F32, tag="r8")
                                    nc.vector.tensor_reduce(
                                        r8[:], e8[:].rearrange("p (t k) -> p t k", k=43),
                                        AXX, ADD)
                                    nc.vector.reciprocal(r8[:], r8[:])
                                    nc.vector.tensor_tensor(
                                        PR[g][j][:].rearrange("p (t k) -> p t k", k=43),
                                        e8[:].rearrange("p (t k) -> p t k", k=43),
                                        r8[:].rearrange("p (t k) -> p t k", k=1)
                                            .broadcast_to([128, GT, 43]),
                                        MULT)
                            sp = rpsum.tile([8, KO], F32, tag="sps")
                            for t in range(32):
                                j, tj = t // GT, t % GT
                                if it == 0:
                                    rhs_t = P_t(g, t)
                                    lhs = t_selb43
                                else:
                                    tm = scr.tile([128, KO], BF16, tag="tm", bufs=6)
                                    pr_b = (PR[g][j][:, tj * 43 : tj * 43 + 43]
                                            .rearrange("p (k o) -> p k o", o=1)
                                            .broadcast_to([128, 43, 21]))
                                    rp_ = (2 * t + g + 3 * it) % 16
                                    if rp_ < PM_EXP:
                                        # expand probs on Act, then a packed
                                        # bf16 multiply hits DVE 2x_1p mode
                                        pre = scr.tile([128, KO], BF16,
                                                       tag="pre", bufs=3)
                                        nc.scalar.activation(
                                            pre[:].rearrange("p (k o) -> p k o", o=21),
                                            pr_b, ACTF.Copy)
                                        nc.vector.tensor_tensor(
                                            tm[:], P_t(g, t), pre[:], MULT)
                                    elif rp_ < PM_DVE:
                                        nc.vector.tensor_tensor(
                                            tm[:].rearrange("p (k o) -> p k o", o=21),
                                            P_t(g, t).rearrange("p (k o) -> p k o", o=21),
                                            pr_b, MULT)
                                    else:
                                        nc.gpsimd.tensor_tensor(
                                            tm[:].rearrange("p (k o) -> p k o", o=21),
                                            P_t(g, t).rearrange("p (k o) -> p k o", o=21),
                                            pr_b, MULT)
                                    rhs_t = tm[:]
                                    lhs = t_selb
                                nc.tensor.matmul(sp[:, 0:512], lhs[:], rhs_t[:, 0:512],
                                                 start=(t == 0), stop=(t == 31))
                                nc.tensor.matmul(sp[:, 512:KO], lhs[:], rhs_t[:, 512:KO],
                                                 start=(t == 0), stop=(t == 31))
                            nc.scalar.activation(s_g[g][:], sp[:], ACTF.Copy)
                        # squash: out = s * sqrt(sn)/(1+sn)
                        for g in range(2):
                            nc.scalar.activation(sq[g][:], s_g[g][:], ACTF.Square)
                            nc.vector.tensor_reduce(
                                sn[g][:], sq[g][:].rearrange("p (k o) -> p k o", o=21),
                                AXX, ADD)
                            nc.vector.tensor_scalar_add(den[g][:], sn[g][:], 1.0)
                            nc.vector.reciprocal(den[g][:], den[g][:])
                            nc.scalar.activation(phi[g][:], sn[g][:], ACTF.Sqrt)
                            nc.vector.tensor_tensor(phi[g][:], phi[g][:], den[g][:], MULT)
                            tgt = out_f[g] if it == 2 else out_bf[g]
                            nc.vector.tensor_tensor(
                                tgt[:].rearrange("p (k o) -> p k o", o=21),
                                s_g[g][:].rearrange("p (k o) -> p k o", o=21),
                                phi[g][:].rearrange("p (k o) -> p k o", o=1)
                                      .broadcast_to([8, 43, 21]),
                                MULT)
                            if it == 2:
                                nc.sync.dma_start(o_out[g * 8 : g * 8 + 8, :], tgt[:])
                        if it < 2:
                            for g in range(2):
                                rpp = rpsum.tile([128, KO], F32, tag="rep")
                                nc.tensor.matmul(
                                    rpp[:, 0:512], t_selr[:],
                                    out_bf[g][:, 0:512], start=True, stop=True)
                                nc.tensor.matmul(
                                    rpp[:, 512:KO], t_selr[:],
                                    out_bf[g][:, 512:KO], start=True, stop=True)
                                nc.scalar.activation(out_rep[g][:], rpp[:], ACTF.Copy)
                                for j in range(NG):
                                    arg = scr.tile([128, GT * 43], F16, tag="arg",
                                                   name="arg", bufs=2)
                                    # two multiplies (independently scheduled
                                    # on DVE/Pool) fill one double-wide ap so
                                    # a single reduce covers both tiles
                                    for tj in range(0, GT, 2):
                                        t = j * GT + tj
                                        ap2 = scr.tile([128, 2 * KO], BF16,
                                                       tag="ap", bufs=3)
                                        for h in range(2):
                                            me = (nc.vector
                                                  if (2 * (t + h) + g + it) % 16
                                                  < DM_DVE else nc.gpsimd)
                                            me.tensor_tensor(
                                                ap2[:, h * KO : (h + 1) * KO],
                                                P_t(g, t + h), out_rep[g][:], MULT)
                                        a3 = ap2[:].rearrange("p (k o) -> p k o", o=21)
                                        with nc.allow_low_precision("logit delta fp16"):
                                            nc.vector.tensor_reduce(
                                                arg[:, tj * 43 : (tj + 2) * 43],
                                                a3, AXX, ADD)
                                    nc.vector.tensor_tensor(
                                        L[g][j][:], L[g][j][:], arg[:], ADD)
    _spill_extra_waits(nc)
    return nc


_CACHED = {}


def _get_bass():
    if "nc" not in _CACHED:
        _CACHED["nc"] = _build_bass()
    return _CACHED["nc"]


def kernel(**inputs):
    from concourse.bass_utils import run_bass_kernel_spmd

    d = {k: np.asarray(v) for k, v in inputs.items()}
    shared = _prep_shared(d)
    x = np.asarray(d["x"], np.float32)

    nc = _get_bass()
    in_maps = []
    for c in range(NCORES):
        m = dict(shared)
        m["xcol"] = _bf(_im2col(x[c]))
        in_maps.append(m)

    import os
    trace = bool(os.environ.get("DCAPS_TRACE"))
    res = run_bass_kernel_spmd(
        nc, in_maps, core_ids=list(range(NCORES)), trace=trace)
    _CACHED["last_results"] = res
    _CACHED["last_in_maps"] = in_maps

    out = np.empty((NCORES, 4, 4, N_CLASSES, 21), np.float32)
    for c in range(NCORES):
        r = np.asarray(res.results[c]["out"])  # (16, 903)
        for gy in range(4):
            for gx in range(4):
                cell = (gy >> 1) * 8 + (gy & 1) * 4 + gx
                out[c, gy, gx] = r[cell].reshape(N_CLASSES, 21)
    return out

